# revision 21
# baseline (speedup 1.0000x reference)
"""Trainium2 Bass kernel for nn_NodeProcessor (GNN message passing).

  agg = segment_sum(edge_attr, edge_index[0], N)       # scatter-add
  h   = silu([x, agg] @ W1 + b1) @ W2 + b2             # MLP
  out = LayerNorm(h) * gamma + beta + x                # LN + residual

Strategy (8 NeuronCores, no collectives):
  * Host: partition the 50k nodes into 400 blocks of exactly 128 nodes via
    round-based LPT bin packing so each block owns <= 2048 incident edges
    (destination-partitioned graph => scatter is fully core-local).
    Edges are permuted into a dense per-core layout; all heavy traffic is
    sequential DMA on device.
  * Device scatter: per 128-edge tile build a one-hot [edge, dest] matrix on
    the vector engine (iota == dst compare) and matmul-accumulate into a
    2-block PSUM window -> feature-major agg directly (no transposes).
  * Device MLP+LN in feature-major form; LayerNorm statistics via ones-matrix
    matmuls (column sums broadcast across partitions for free).
  * float32r matmuls throughout (full PE speed at N>=256, ~1e-4 rel err).
  * Host: un-permute + transpose the output.
"""

import os
import sys
import types
from dataclasses import dataclass

import numpy as np

import concourse.bacc as bacc
import concourse.mybir as mybir
from concourse import bass_utils
from concourse.masks import make_identity
from concourse.tile import TileContext

# ---------------------------------------------------------------- constants
P = 128
D = 256                      # node/edge feature dim
H = 512                      # MLP hidden dim
LN_EPS = 1e-5
NDEST = P                    # scatter window = one 128-node block

F32 = mybir.dt.float32
F32R = mybir.dt.float32r
F16 = mybir.dt.float16

_TRACE = os.environ.get("KERNEL_TRACE", "0") == "1"


@dataclass(frozen=True)
class Cfg:
    n_nodes: int = 50000
    n_cores: int = 8
    bpc: int = 50            # blocks per core (must be even)
    tpb: int = 16            # edge tiles per block (tpb*128 edge slots)
    group: int = 512         # MLP moving-dim node group

    @property
    def npc(self):           # nodes per core
        return self.bpc * P

    @property
    def nbins(self):
        return self.n_cores * self.bpc

    @property
    def npad(self):
        return self.nbins * P

    @property
    def pairs(self):
        return self.bpc // 2


FULL = Cfg()


def _install_ntff_hook():
    try:
        from antenv.axon_hooks import get_axon_ntff_profile_hook  # noqa: F401
        return
    except ImportError:
        pass
    try:
        import antenv
        from trn_agent_boot.trn_boot import _ntff_profile_via_ctypes
    except ImportError:
        return
    mod = types.ModuleType("antenv.axon_hooks")
    state = {"hook": _ntff_profile_via_ctypes("/opt/axon/libaxon_pjrt.so")}
    mod.set_axon_ntff_profile_hook = lambda h: state.__setitem__("hook", h)
    mod.get_axon_ntff_profile_hook = lambda: state["hook"]
    sys.modules["antenv.axon_hooks"] = mod
    antenv.axon_hooks = mod


if _TRACE:
    _install_ntff_hook()
    bass_utils.upload_artifacts = lambda tmpdir: "local://" + str(tmpdir)


# ---------------------------------------------------------------- device IR
def build_program(cfg: Cfg):
    """Build the per-core Bass program. All cores run the same NEFF with
    different input bindings."""
    nc = bacc.Bacc("TRN2", target_bir_lowering=False, debug=False,
                   num_devices=cfg.n_cores)

    tpb, bpc, npc, group = cfg.tpb, cfg.bpc, cfg.npc, cfg.group
    ecols = tpb * D
    e_d = nc.dram_tensor("e_in", [npc, ecols], F16, kind="ExternalInput")
    dst_d = nc.dram_tensor("dstloc_in", [P, bpc * tpb], F16, kind="ExternalInput")
    xt16_d = nc.dram_tensor("xt16_in", [2 * P, npc], F16, kind="ExternalInput")
    iota_d = nc.dram_tensor("iota_in", [P, NDEST], F16, kind="ExternalInput")
    ones_d = nc.dram_tensor("ones_in", [P, P], F16, kind="ExternalInput")
    w1_d = nc.dram_tensor("w1_in", [H, H], F16, kind="ExternalInput")
    w2_d = nc.dram_tensor("w2_in", [H, D], F16, kind="ExternalInput")
    b1_d = nc.dram_tensor("b1c_in", [P, 4], F32, kind="ExternalInput")
    b2_d = nc.dram_tensor("b2c_in", [P, 2], F32, kind="ExternalInput")
    gam_d = nc.dram_tensor("gammac_in", [P, 2], F32, kind="ExternalInput")
    bet_d = nc.dram_tensor("betac_in", [P, 2], F32, kind="ExternalInput")
    out_d = nc.dram_tensor("outT", [2 * P, npc], F32, kind="ExternalOutput")

    # MLP node groups: (start, size)
    groups = []
    n0 = 0
    while n0 < npc:
        nn = min(group, npc - n0)
        groups.append((n0, nn))
        n0 += nn

    _dbg = {}
    with TileContext(nc) as tc:
        with tc.tile_pool(name="const", bufs=1) as cpool, \
             tc.tile_pool(name="edges", bufs=5) as epool, \
             tc.tile_pool(name="onehot", bufs=5) as mpool, \
             tc.tile_pool(name="aggbuf", bufs=1) as apool, \
             tc.tile_pool(name="work", bufs=3) as wpool, \
             tc.tile_pool(name="ln", bufs=3) as lpool, \
             tc.tile_pool(name="ps_sc", bufs=2, space="PSUM") as scpool, \
             tc.tile_pool(name="ps_mlp", bufs=4, space="PSUM") as mlppool:

            # ---- constants ------------------------------------------------
            iota_s = cpool.tile([P, NDEST], F16)
            ones_s = cpool.tile([P, P], F16)
            ident_s = cpool.tile([P, P], F16)
            make_identity(nc, ident_s[:])
            dst_s = cpool.tile([P, bpc * tpb], F16)
            w1_s = cpool.tile([P, 4 * H], F16)
            w2_s = cpool.tile([P, 4 * D], F16)
            b1_s = cpool.tile([P, 4], F32)
            b2_s = cpool.tile([P, 2], F32)
            gam_s = cpool.tile([P, 2], F32)
            bet_s = cpool.tile([P, 2], F32)
            eps_s = cpool.tile([P, 1], F32)
            nc.gpsimd.memset(eps_s[:], LN_EPS)
            nc.sync.dma_start(out=iota_s[:], in_=iota_d.ap())
            nc.sync.dma_start(out=dst_s[:], in_=dst_d.ap())

            def load_mlp_consts():
                nc.sync.dma_start(out=ones_s[:], in_=ones_d.ap())
                for k in range(4):
                    nc.sync.dma_start(out=w1_s[:, k * H:(k + 1) * H],
                                      in_=w1_d.ap()[k * P:(k + 1) * P, :])
                    nc.sync.dma_start(out=w2_s[:, k * D:(k + 1) * D],
                                      in_=w2_d.ap()[k * P:(k + 1) * P, :])
                nc.sync.dma_start(out=b1_s[:], in_=b1_d.ap())
                nc.sync.dma_start(out=b2_s[:], in_=b2_d.ap())
                nc.sync.dma_start(out=gam_s[:], in_=gam_d.ap())
                nc.sync.dma_start(out=bet_s[:], in_=bet_d.ap())

            # feature-major aggregation buffer (persistent)
            agg0_s = apool.tile([P, npc], F16)    # features 0..127
            agg1_s = apool.tile([P, npc], F16)    # features 128..255
            agg_half = [agg0_s, agg1_s]

            # ---- phase work, interleaved per MLP group --------------------
            def scatter_block(b):
                """Scatter-add one block's 16 edge tiles -> dest-major SBUF
                tile (returned for a later batched transpose)."""
                dm_ps = scpool.tile([P, D], F32, tag="sc", space="PSUM")
                e_s = epool.tile([P, ecols], F16, tag="e")
                if b == 0:
                    # split the first edge DMA so PE can start sooner
                    for q in range(4):
                        nc.sync.dma_start(
                            out=e_s[:, q * (ecols // 4):(q + 1) * (ecols // 4)],
                            in_=e_d.ap()[:P, q * (ecols // 4):(q + 1) * (ecols // 4)])
                else:
                    nc.sync.dma_start(out=e_s[:],
                                      in_=e_d.ap()[b * P:(b + 1) * P, :])
                # one-hot matrices for all tpb tiles of this block in ONE
                # DVE/GpSimd op: out[p, (t, d)] = (iota[d] == dst[p, b*tpb+t])
                m_s = mpool.tile([P, tpb * NDEST], F16, tag="m")
                iota_bc = iota_s[:].rearrange("p (o d) -> p o d", o=1).to_broadcast([P, tpb, NDEST])
                dst_bc = dst_s[:, b * tpb:(b + 1) * tpb].rearrange("p (t o) -> p t o", o=1).to_broadcast([P, tpb, NDEST])
                nc.vector.tensor_tensor(
                    out=m_s[:].rearrange("p (t d) -> p t d", t=tpb),
                    in0=iota_bc, in1=dst_bc,
                    op=mybir.AluOpType.is_equal,
                )
                for t in range(tpb):
                    nc.tensor.matmul(
                        out=dm_ps[:],
                        lhsT=m_s[:, t * NDEST:(t + 1) * NDEST],
                        rhs=e_s[:, t * D:(t + 1) * D],
                        start=(t == 0), stop=(t == tpb - 1),
                    )
                dm_sb = wpool.tile([P, D], F16, tag="dm", bufs=6)
                nc.scalar.copy(out=dm_sb[:], in_=dm_ps[:])
                return dm_sb

            def transpose_block(b, dm_sb):
                for f in range(2):
                    tp_ps = scpool.tile([P, P], F16, tag="tp", space="PSUM")
                    nc.tensor.transpose(
                        out=tp_ps[:], in_=dm_sb[:, f * P:(f + 1) * P],
                        identity=ident_s[:])
                    nc.scalar.copy(
                        out=agg_half[f][:, b * P:(b + 1) * P], in_=tp_ps[:])

            def mlp_group(n0, nn):
                # x chunks for this group: [128, nn] x2 (features 0:128, 128:256)
                xt16_s = wpool.tile([P, 2 * group], F16, tag="xt16", bufs=3)
                _dbg.setdefault("xt", []).append(xt16_s)
                for c in range(2):
                    nc.sync.dma_start(
                        out=xt16_s[:, c * group: c * group + nn],
                        in_=xt16_d.ap()[c * P:(c + 1) * P, n0:n0 + nn])
                rhs_k = [
                    xt16_s[:, 0:nn],
                    xt16_s[:, group:group + nn],
                    agg0_s[:, n0:n0 + nn],
                    agg1_s[:, n0:n0 + nn],
                ]
                # hT = silu(W1.T @ [x; agg] + b1)  -> [512(4x128), nn]
                h_s = wpool.tile([P, 4 * group], F16, tag="h")
                _dbg.setdefault("h", []).append(h_s)
                for j in range(4):
                    hps = mlppool.tile([P, group], F32, tag="mm", space="PSUM")
                    for k in range(4):
                        nc.tensor.matmul(
                            out=hps[:, :nn],
                            lhsT=w1_s[:, k * H + j * P: k * H + (j + 1) * P],
                            rhs=rhs_k[k],
                            start=(k == 0), stop=(k == 3),
                        )
                    nc.scalar.activation(
                        out=h_s[:, j * group: j * group + nn], in_=hps[:, :nn],
                        func=mybir.ActivationFunctionType.Silu,
                        bias=b1_s[:, j:j + 1])
                # yT = W2.T @ hT + b2 -> [256(2x128), nn]; LN stats via ones
                y_s = [None, None]
                sps = mlppool.tile([P, group], F32, tag="mm", space="PSUM")
                for m in range(2):
                    yps = mlppool.tile([P, group], F32, tag="mm", space="PSUM")
                    for j in range(4):
                        nc.tensor.matmul(
                            out=yps[:, :nn],
                            lhsT=w2_s[:, j * D + m * P: j * D + (m + 1) * P],
                            rhs=h_s[:, j * group: j * group + nn],
                            start=(j == 0), stop=(j == 3),
                        )
                    ym = lpool.tile([P, group], F16, tag="y", bufs=3)
                    _dbg.setdefault("y", []).append(ym)
                    nc.scalar.activation(
                        out=ym[:, :nn], in_=yps[:, :nn],
                        func=mybir.ActivationFunctionType.Identity,
                        bias=b2_s[:, m:m + 1])
                    y_s[m] = ym
                    # column sums (broadcast to all partitions via ones matrix)
                    nc.tensor.matmul(out=sps[:, :nn], lhsT=ones_s[:],
                                     rhs=ym[:, :nn],
                                     start=(m == 0), stop=(m == 1))
                # sps already holds mean (ones matrix is scaled by 1/D)
                # t = y - mu ; q = mean(t^2) ; rstd = 1/sqrt(q + eps)
                qps = mlppool.tile([P, group], F32, tag="mm", space="PSUM")
                t_s = [None, None]
                for m in range(2):
                    tm = lpool.tile([P, group], F32, tag="t", bufs=3)
                    nc.vector.tensor_sub(out=tm[:, :nn],
                                         in0=y_s[m][:, :nn],
                                         in1=sps[:, :nn])
                    t_s[m] = tm
                    sq = lpool.tile([P, group], F16, tag="sq")
                    nc.scalar.activation(out=sq[:, :nn], in_=tm[:, :nn],
                                         func=mybir.ActivationFunctionType.Square)
                    nc.tensor.matmul(out=qps[:, :nn], lhsT=ones_s[:],
                                     rhs=sq[:, :nn],
                                     start=(m == 0), stop=(m == 1))
                std_s = lpool.tile([P, group], F32, tag="std")
                nc.scalar.activation(out=std_s[:, :nn], in_=qps[:, :nn],
                                     func=mybir.ActivationFunctionType.Sqrt,
                                     scale=1.0, bias=eps_s[:, 0:1])
                rstd_s = lpool.tile([P, group], F32, tag="rstd")
                nc.vector.reciprocal_approx_fast(out=rstd_s[:, :nn],
                                                 in_=std_s[:, :nn])
                for m in range(2):
                    um = lpool.tile([P, group], F32, tag="u")
                    # (t * gamma) * rstd
                    nc.vector.scalar_tensor_tensor(
                        out=um[:, :nn], in0=t_s[m][:, :nn],
                        scalar=gam_s[:, m:m + 1],
                        in1=rstd_s[:, :nn],
                        op0=mybir.AluOpType.mult,
                        op1=mybir.AluOpType.mult)
                    om = lpool.tile([P, group], F32, tag="o")
                    # (u + beta) + xT
                    nc.vector.scalar_tensor_tensor(
                        out=om[:, :nn], in0=um[:, :nn],
                        scalar=bet_s[:, m:m + 1],
                        in1=xt16_s[:, m * group: m * group + nn],
                        op0=mybir.AluOpType.add,
                        op1=mybir.AluOpType.add)
                    nc.sync.dma_start(
                        out=out_d.ap()[m * P:(m + 1) * P, n0:n0 + nn],
                        in_=om[:, :nn])

            # interleave: scatter blocks needed by each group, then the group
            blk = 0
            ngrp = len(groups)
            for gi, (n0, nn) in enumerate(groups):
                # front-load scatter so the tail group's blocks finish early
                target = bpc if gi >= ngrp - 2 else -(-bpc * (gi + 1) // (ngrp - 1))
                target = max(target, (n0 + nn + P - 1) // P)
                while blk < min(target, bpc):
                    transpose_block(blk, scatter_block(blk))
                    blk += 1
                    if blk == 1:
                        load_mlp_consts()
                mlp_group(n0, nn)

    nc.debug_aps = {"agg0": agg0_s, "agg1": agg1_s,
                    "iota": iota_s, "dst": dst_s, **_dbg}
    nc.compile()
    return nc


_NC_CACHE = {}


def _get_program(cfg: Cfg):
    if cfg not in _NC_CACHE:
        _NC_CACHE[cfg] = build_program(cfg)
    return _NC_CACHE[cfg]


# ---------------------------------------------------------- host preprocessing
def _pack_nodes(deg, cfg: Cfg):
    """Assign each node (incl. padding) to one of nbins bins: exactly P nodes
    per bin, minimizing max edge load (round-based LPT). Returns (bin_of,
    slot_of, max_load)."""
    nbins = cfg.nbins
    order = np.argsort(-deg, kind="stable")
    bin_of = np.empty(cfg.npad, np.int32)
    slot_of = np.empty(cfg.npad, np.int32)
    loads = np.zeros(nbins, np.int64)
    for r in range(P):
        grp = order[r * nbins:(r + 1) * nbins]
        which = np.argsort(loads, kind="stable")
        bin_of[grp] = which
        slot_of[grp] = r
        loads[which] += deg[grp]
    return bin_of, slot_of, int(loads.max())


def preprocess(x, edge_index, edge_attr, cfg: Cfg):
    dst = np.asarray(edge_index[0], dtype=np.int64).astype(np.int32)
    deg = np.bincount(dst, minlength=cfg.npad).astype(np.int64)
    bin_of, slot_of, max_load = _pack_nodes(deg, cfg)

    tpb = cfg.tpb
    while max_load > tpb * P:
        tpb += 1  # robustness fallback; never triggers for the spec'd sizes
    if tpb != cfg.tpb:
        cfg = Cfg(cfg.n_nodes, cfg.n_cores, cfg.bpc, tpb, cfg.group)

    cap = tpb * P
    nbins, bpc, npad = cfg.nbins, cfg.bpc, cfg.npad
    # order edges by destination bin
    ebin = bin_of[dst]
    eorder = np.argsort(ebin, kind="stable")
    ebin_sorted = ebin[eorder]
    counts = np.bincount(ebin_sorted, minlength=nbins)
    offs = np.zeros(nbins + 1, np.int64)
    np.cumsum(counts, out=offs[1:])
    pos = np.arange(len(eorder), dtype=np.int64) - offs[ebin_sorted]
    gslot = ebin_sorted * cap + pos

    slot_edge = np.zeros(nbins * cap, np.int64)
    slot_dst = np.full(nbins * cap, -1.0, np.float16)
    slot_edge[gslot] = eorder
    slot_dst[gslot] = slot_of[dst[eorder]].astype(np.float16)

    # node id at [bin, slot]
    node_at = np.empty((nbins, P), np.int32)
    node_at[bin_of[np.arange(npad)], slot_of[np.arange(npad)]] = \
        np.arange(npad, dtype=np.int32)

    x_pad = np.zeros((npad, D), np.float32)
    x_pad[:cfg.n_nodes] = x
    xb_pad = x_pad

    e_src = np.asarray(edge_attr).astype(np.float16)

    per_core = []
    for c in range(cfg.n_cores):
        sl = slice(c * bpc * cap, (c + 1) * bpc * cap)
        ids = slot_edge[sl]
        e_core = e_src[ids]                                   # [bpc*cap, D]
        # [bin, p, t, D] with slot s = p*tpb + t  ->  [bpc*P, tpb*D]
        e_core = np.ascontiguousarray(e_core.reshape(bpc * P, tpb * D))
        dl = slot_dst[sl].reshape(bpc, P, tpb)
        dl = np.ascontiguousarray(dl.transpose(1, 0, 2).reshape(P, bpc * tpb))
        nodes_c = node_at[c * bpc:(c + 1) * bpc].reshape(-1)  # [npc]
        xtb = np.ascontiguousarray(xb_pad[nodes_c].T)         # [D, npc]
        xt16 = xtb.astype(np.float16)
        per_core.append((e_core, dl, xtb, nodes_c, xt16))
    return per_core, cfg


def make_shared_inputs(W1, W2, b1, b2, gamma, beta):
    iota = np.tile(np.arange(NDEST, dtype=np.float16), (P, 1))
    ones = np.full((P, P), 1.0 / D, np.float16)
    b1c = np.ascontiguousarray(np.asarray(b1, np.float32).reshape(4, P).T)
    b2c = np.ascontiguousarray(np.asarray(b2, np.float32).reshape(2, P).T)
    gammac = np.ascontiguousarray(np.asarray(gamma, np.float32).reshape(2, P).T)
    betac = np.ascontiguousarray(np.asarray(beta, np.float32).reshape(2, P).T)
    return {
        "iota_in": iota, "ones_in": ones,
        "w1_in": np.asarray(W1, np.float32).astype(np.float16),
        "w2_in": np.asarray(W2, np.float32).astype(np.float16),
        "b1c_in": b1c, "b2c_in": b2c, "gammac_in": gammac, "betac_in": betac,
    }


# ------------------------------------------------------------------- kernel
def kernel(x, edge_index, edge_attr, W1, b1, W2, b2, gamma, beta):
    x = np.asarray(x, np.float32)

    per_core, cfg = preprocess(x, edge_index, edge_attr, FULL)
    nc = _get_program(cfg)

    shared = make_shared_inputs(W1, W2, b1, b2, gamma, beta)
    in_maps = []
    for (e_core, dl, xtb, _nodes, xt16) in per_core:
        m = dict(shared)
        m["e_in"] = e_core
        m["dstloc_in"] = dl
        m["xt16_in"] = xt16
        in_maps.append(m)

    res = bass_utils.run_bass_kernel_spmd(
        nc, in_maps, core_ids=list(range(cfg.n_cores)), trace=_TRACE,
    )
    if _TRACE and res.exec_time_ns is not None:
        print(f"HW exec time: {res.exec_time_ns} ns")
        if res.instructions_and_trace is not None:
            print("trace:", res.instructions_and_trace[1])

    out = np.empty((cfg.n_nodes, D), np.float32)
    for c in range(cfg.n_cores):
        nodes_c = per_core[c][3]
        valid = nodes_c < cfg.n_nodes
        outT = res.results[c]["outT"]                         # [D, npc]
        out[nodes_c[valid]] = outT[:, valid].T
    return out
